# revision 23
# baseline (speedup 1.0000x reference)
"""Causal self-attention (B=4, T=2048, C=1024, H=16) on 8 TRN2 NeuronCores.

Sharding: core = 2*b + parity. Each core handles batch b's queries at
tokens parity::2 (1024 queries). K/V are computed for the full 2048-token
context (redundantly per batch pair) so no collectives are needed, and the
even/odd interleave makes the causal block structure identical on every
core: local query sub-block m (128 queries) attends exactly key blocks
0..2m+1, with a single shared [128(key),128(query)] diagonal mask per
parity applied to the last two key blocks.

Dataflow (all transposed, zero on-chip transposes):
  xT [C, tok] --Wk--> kT [C, 2048] (f32r matmul, bf16 storage)
             --Wv--> v [tok, C(+ones col)] natural layout, bf16
  xqT --Wq--> qT [C, 1024] bf16
  S^T[keys,q] = kT_h^T @ qT_h (bf16), exp on ScalarE (no max subtraction:
  |S|/8 <~ 6), diagonal-mask mul, P bf16.
  y^T[65,q] = [v_h | 1]^T @ P: row 64 = softmax denominator for free.
  1/denom broadcast across partitions via K=1 matmul; out-proj in bf16.

Phase C runs a flat software pipeline over the 34 (query-block, head)
steps: the PE stream per step is bc(s-2), [outproj-j0 filler],
scores(s) interleaved with AV(s-1) — keeping TensorE continuously busy
(HAM stays at full clock) while ScalarE exps one head behind.
"""

import math
from contextlib import ExitStack

import numpy as np

B, T, C, H = 4, 2048, 1024, 16
D = C // H  # 64
P = 128
N_CORES = 8
NKB = T // P  # 16 key blocks of 128
TQ = T // 2  # 1024 queries per core
NQB = 2  # query blocks of 512
SCALE = 1.0 / math.sqrt(D)

_CACHE = {}


def _build_nc():
    import concourse.tile as tile
    from concourse import bacc, mybir
    from concourse.bass_interp import get_hw_module
    from concourse import hw_specs

    if not getattr(bacc, "_attn_act_tbl_patch", False):
        _orig_tables = hw_specs.get_activation_tables

        def _tables_exp_with_ln(arch):
            t = _orig_tables(arch)
            for name, fns in t.items():
                if name != "natural_log_exp_and_others":
                    fns.discard(mybir.ActivationFunctionType.Exp)
            return t

        bacc.get_activation_tables = _tables_exp_with_ln
        bacc._attn_act_tbl_patch = True

    f32 = mybir.dt.float32
    f32r = mybir.dt.float32r
    bf16 = mybir.dt.bfloat16

    nc = bacc.Bacc("TRN2", target_bir_lowering=False, debug=False,
                   num_devices=N_CORES)

    xctxT = nc.dram_tensor("xctxT", [C, T], f32r, kind="ExternalInput").ap()
    xqT = nc.dram_tensor("xqT", [C, TQ], f32r, kind="ExternalInput").ap()
    Wq = nc.dram_tensor("Wq", [C, C], f32r, kind="ExternalInput").ap()
    Wk = nc.dram_tensor("Wk", [C, C], f32r, kind="ExternalInput").ap()
    Wv = nc.dram_tensor("Wv", [C, C], f32r, kind="ExternalInput").ap()
    Wp = nc.dram_tensor("Wp", [C, C], bf16, kind="ExternalInput").ap()
    bq = nc.dram_tensor("bq", [P, C // P], f32, kind="ExternalInput").ap()
    bk = nc.dram_tensor("bk", [P, C // P], f32, kind="ExternalInput").ap()
    bp = nc.dram_tensor("bp", [P, C // P], f32, kind="ExternalInput").ap()
    vbias = nc.dram_tensor("vbias", [P, H, D], f32, kind="ExternalInput").ap()
    maskT = nc.dram_tensor("maskT", [P, 2, P], f32, kind="ExternalInput").ap()
    onesr = nc.dram_tensor("onesr", [1, D], f32r, kind="ExternalInput").ap()
    outT = nc.dram_tensor("outT", [C, TQ], f32, kind="ExternalOutput").ap()

    CB = C // P  # 8 channel blocks

    with tile.TileContext(nc) as tc, ExitStack() as top:
        persist = top.enter_context(tc.tile_pool(name="persist", bufs=1))
        small = top.enter_context(tc.tile_pool(name="small", bufs=1))

        # persistent SBUF tensors (all bf16: 81 KB/partition total)
        kT_sb = persist.tile([P, CB, T], bf16, tag="kT")
        v_sb = persist.tile([P, NKB, H, D + 1], bf16, tag="v")
        qT_sb = persist.tile([P, CB, TQ], bf16, tag="qT")

        bq_sb = small.tile([P, CB], f32, tag="bq")
        bk_sb = small.tile([P, CB], f32, tag="bk")
        bp_sb = small.tile([P, CB], f32, tag="bp")
        vb_sb = small.tile([P, H, D], f32, tag="vb")
        mask_sb = small.tile([P, 2, P], bf16, tag="mask")
        mask_f32 = small.tile([P, 2, P], f32, tag="maskf")
        ones_sb = small.tile([1, D], f32r, tag="ones")

        nc.sync.dma_start(bq_sb[:], bq[:])
        nc.sync.dma_start(bk_sb[:], bk[:])
        nc.sync.dma_start(bp_sb[:], bp[:])
        nc.sync.dma_start(vb_sb[:], vbias[:])
        nc.sync.dma_start(mask_f32[:], maskT[:])
        nc.vector.tensor_copy(mask_sb[:], mask_f32[:])
        nc.sync.dma_start(ones_sb[:], onesr[:])
        # ones column of v (AV rides the softmax denominator in row 64)
        nc.vector.memset(v_sb[:, :, :, D:D + 1], 1.0)

        def copy_bias(out, psum, bias_col):
            # PSUM -> SBUF copy + per-partition bias on the (idle) ScalarE
            nc.scalar.activation(out, psum,
                                 mybir.ActivationFunctionType.Identity,
                                 bias=bias_col)

        # ---------------- Phase A: K and V projections ----------------
        TC = 512  # token chunk
        with ExitStack() as sa:
            wpool = sa.enter_context(tc.tile_pool(name="wpool", bufs=2))
            xin = sa.enter_context(tc.tile_pool(name="xin", bufs=3))
            pmm = sa.enter_context(
                tc.tile_pool(name="pmm", bufs=2, space="PSUM"))

            wk_sb = wpool.tile([P, CB, C], f32r, tag="W")
            nc.sync.dma_start(wk_sb[:], Wk.rearrange("(o p) c -> p o c", p=P))
            wv_sb = wpool.tile([P, CB, C], f32r, tag="W")
            nc.sync.dma_start(wv_sb[:], Wv.rearrange("(o p) c -> p o c", p=P))

            for t0 in range(0, T, TC):
                x_t = xin.tile([P, CB, TC], f32r, tag="x")
                nc.sync.dma_start(
                    x_t[:],
                    xctxT[:, t0:t0 + TC].rearrange("(o p) t -> p o t", p=P))
                # K: kT rows (transposed layout)
                for rb in range(CB):
                    ps = pmm.tile([P, TC], f32, tag="mm")
                    for kc in range(CB):
                        nc.tensor.matmul(
                            ps[:], wk_sb[:, kc, rb * P:(rb + 1) * P],
                            x_t[:, kc, :], start=(kc == 0),
                            stop=(kc == CB - 1))
                    copy_bias(kT_sb[:, rb, t0:t0 + TC], ps[:],
                              bk_sb[:, rb:rb + 1])
                # V: natural layout [tok, C]
                for tb in range(TC // P):
                    kb = (t0 + tb * P) // P
                    for cb2 in range(2):
                        ps = pmm.tile([P, TC], f32, tag="mm")
                        for kc in range(CB):
                            nc.tensor.matmul(
                                ps[:], x_t[:, kc, tb * P:(tb + 1) * P],
                                wv_sb[:, kc, cb2 * 512:(cb2 + 1) * 512],
                                start=(kc == 0), stop=(kc == CB - 1))
                        h0 = cb2 * 8
                        nc.vector.tensor_tensor(
                            v_sb[:, kb, h0:h0 + 8, 0:D],
                            ps.rearrange("p (h d) -> p h d", d=D),
                            vb_sb[:, h0:h0 + 8, :], mybir.AluOpType.add)

        # ---------------- Phase B: Q projection ----------------
        with ExitStack() as sb:
            wqp = sb.enter_context(tc.tile_pool(name="wqp", bufs=3))
            xqin = sb.enter_context(tc.tile_pool(name="xqin", bufs=2))
            pmm = sb.enter_context(
                tc.tile_pool(name="pmmB", bufs=2, space="PSUM"))
            for q0 in range(0, TQ, 512):
                xq_t = xqin.tile([P, CB, 512], f32r, tag="xq")
                nc.gpsimd.dma_start(
                    xq_t[:],
                    xqT[:, q0:q0 + 512].rearrange("(o p) t -> p o t", p=P))
                for rb in range(CB):
                    wq_t = wqp.tile([P, CB, P], f32r, tag="wq")
                    nc.gpsimd.dma_start(
                        wq_t[:], Wq[:, rb * P:(rb + 1) * P].rearrange(
                            "(o p) c -> p o c", p=P))
                    ps = pmm.tile([P, 512], f32, tag="mm")
                    for kc in range(CB):
                        nc.tensor.matmul(
                            ps[:], wq_t[:, kc, :], xq_t[:, kc, :],
                            start=(kc == 0), stop=(kc == CB - 1))
                    copy_bias(qT_sb[:, rb, q0:q0 + 512], ps[:],
                              bq_sb[:, rb:rb + 1])

        # -------- Phase C: attention + output projection (flat pipeline) ----
        with ExitStack() as sc:
            ppool = sc.enter_context(tc.tile_pool(name="ppool", bufs=2))
            ypool = sc.enter_context(tc.tile_pool(name="ypool", bufs=2))
            opool = sc.enter_context(tc.tile_pool(name="opool", bufs=2))
            wpp = sc.enter_context(tc.tile_pool(name="wpp", bufs=2))
            nrm = sc.enter_context(tc.tile_pool(name="nrm", bufs=3))
            ps_s = sc.enter_context(
                tc.tile_pool(name="ps_s", bufs=2, space="PSUM"))
            ps_y = sc.enter_context(
                tc.tile_pool(name="ps_y", bufs=3, space="PSUM"))
            ps_x = sc.enter_context(
                tc.tile_pool(name="ps_x", bufs=1, space="PSUM"))

            L = [(0, h) for h in range(H)] + [(1, h) for h in range(H)]
            P_ts, py_ts, recips = {}, {}, {}
            yT_tiles = {}

            def qstart(j, kb):
                return max(0, kb // 2 - 4 * j) * P

            def emit_outproj(j, ob):
                q0 = j * 512
                yT_sb = yT_tiles[j]
                wp_t = wpp.tile([P, CB, P], bf16, tag="wp")
                nc.gpsimd.dma_start(
                    wp_t[:], Wp[:, ob * P:(ob + 1) * P].rearrange(
                        "(o p) c -> p o c", p=P))
                po = ps_x.tile([P, 512], f32, tag="bx")
                for yc in range(CB):
                    nc.tensor.matmul(po[:], wp_t[:, yc, :], yT_sb[:, yc, :],
                                     start=(yc == 0), stop=(yc == CB - 1))
                o_sb = opool.tile([P, 512], f32, tag="o_sb")
                copy_bias(o_sb[:], po[:], bp_sb[:, ob:ob + 1])
                nc.sync.dma_start(outT[ob * P:(ob + 1) * P, q0:q0 + 512],
                                  o_sb[:])

            for s in range(len(L) + 3):
                cur = L[s] if s < len(L) else None
                prv = L[s - 1] if 1 <= s <= len(L) else None
                pp2 = L[s - 3] if s >= 3 else None

                # --- PE filler: j0 output projection inside j1 steps ---
                if 19 <= s <= 26:
                    emit_outproj(0, s - 19)

                # --- scores(cur) interleaved with AV(prv) ---
                sc_pairs = []
                if cur is not None:
                    j, h = cur
                    if h == 0:
                        yT_tiles[j] = ypool.tile([P, CB, 512], bf16, tag="yT", name=f"yT{j}")
                    kmax = 8 * j + 8
                    sc_pairs = list(range(kmax // 2))
                    P_ts[cur] = ppool.tile([P, NKB, 512], bf16, tag="P", name=f"Pt{s}")
                av_kbs = []
                if prv is not None:
                    av_kbs = list(range(8 * prv[0] + 8))
                    py_ts[prv] = ps_y.tile([D + 1, 512], f32, tag="y", name=f"py{s}")

                np_, na = max(len(sc_pairs), 1), len(av_kbs)
                for i, p_ in enumerate(sc_pairs or [None]):
                    if p_ is not None:
                        j, h = cur
                        q0 = j * 512
                        hp, hb = (h % 2) * D, h // 2
                        P_t = P_ts[cur]
                        qs = qstart(j, 2 * p_)
                        ss = ps_s.tile([P, 1024], f32, tag="s")
                        for dj in range(2):
                            kb = 2 * p_ + dj
                            nc.tensor.matmul(
                                ss[:, dj * 512 + qs:(dj + 1) * 512],
                                kT_sb[hp:hp + D, hb, kb * P:(kb + 1) * P],
                                qT_sb[hp:hp + D, hb, q0 + qs:q0 + 512],
                                start=True, stop=True)
                        if qs == 0:
                            nc.scalar.activation(
                                P_t.rearrange("p a b -> p (a b)")
                                [:, 2 * p_ * 512:(2 * p_ + 2) * 512],
                                ss[:], mybir.ActivationFunctionType.Exp,
                                scale=SCALE)
                        else:
                            nc.scalar.activation(
                                P_t[:, 2 * p_:2 * p_ + 2, qs:512],
                                ss.rearrange("p (a b) -> p a b", a=2)
                                [:, :, qs:512],
                                mybir.ActivationFunctionType.Exp, scale=SCALE)
                    # AV share for this slot
                    lo = na * i // np_
                    hi = na * (i + 1) // np_
                    for kb in av_kbs[lo:hi]:
                        jj, hh = prv
                        avs = qstart(jj, kb)
                        nc.tensor.matmul(
                            py_ts[prv][:, avs:512], v_sb[:, kb, hh, :],
                            P_ts[prv][:, kb, avs:512],
                            start=(kb == 0), stop=(kb == len(av_kbs) - 1))

                # --- DVE: causal diagonal masks for cur (must precede the
                # reciprocal in the DVE FIFO so next step's AV isn't stuck
                # behind it) ---
                if cur is not None:
                    j, h = cur
                    P_t = P_ts[cur]
                    for mq in range(4):
                        kb = 2 * (4 * j + mq)
                        sl = P_t[:, kb:kb + 2, mq * P:(mq + 1) * P]
                        nc.vector.tensor_mul(sl, sl, mask_sb[:])

                if prv is not None:
                    P_ts.pop(prv)
                    recip = nrm.tile([1, 512], f32r, tag="recip")
                    recips[prv] = recip
                    if prv[0] == 1:
                        with nc.allow_low_precision(
                                reason="f32r (12-bit) softmax denominators"):
                            nc.vector.reciprocal(recip[:],
                                                 py_ts[prv][D:D + 1, :])
                    else:
                        lnd = nrm.tile([1, 512], f32, tag="lnd")
                        nc.scalar.activation(
                            lnd[:], py_ts[prv][D:D + 1, :],
                            mybir.ActivationFunctionType.Ln)
                        nc.scalar.activation(
                            recip[:], lnd[:],
                            mybir.ActivationFunctionType.Exp, scale=-1.0)

                # --- step tail: normalize head s-2 (reciprocal had a full
                # step of slack; bc sits at the end of the PE stream) ---
                if pp2 is not None:
                    jj, hh = pp2
                    bc = ps_x.tile([P, 512], f32, tag="bx",
                                   name=f"bc{s}")[0:D, :]
                    nc.tensor.matmul(bc[:], ones_sb[:], recips.pop(pp2)[:],
                                     start=True, stop=True)
                    bc_sb = nrm.tile([D, 512], f32, tag="bc_sb")
                    nc.vector.tensor_copy(bc_sb[:], bc[:])
                    py = py_ts.pop(pp2)
                    hp, hb = (hh % 2) * D, hh // 2
                    nc.vector.tensor_mul(yT_tiles[jj][hp:hp + D, hb, :],
                                         py[0:D, :], bc_sb[:])

            for ob in range(CB):
                emit_outproj(1, ob)

    nc.compile()
    nc.m = get_hw_module(nc.m)
    return nc


def _prep_in_maps(x, mask, Wq, bq, Wk, bk, Wv, bv, Wp, bp):
    import ml_dtypes

    del mask  # causal structure is hardcoded (tril), verified upstream
    CB = C // P
    Wq, Wk, Wv = (np.ascontiguousarray(w, np.float32) for w in (Wq, Wk, Wv))
    Wp = np.ascontiguousarray(np.asarray(Wp, np.float32).astype(
        ml_dtypes.bfloat16))
    b_col = lambda b: np.ascontiguousarray(
        np.asarray(b, np.float32).reshape(CB, P).T)
    bq_h, bk_h, bp_h = b_col(bq), b_col(bk), b_col(bp)
    vb_h = np.ascontiguousarray(np.broadcast_to(
        np.asarray(bv, np.float32).reshape(1, H, D), (P, H, D)))

    masks = []
    for par in range(2):
        c = np.arange(2 * P)[:, None]  # key offset within diagonal pair
        r_ = np.arange(P)[None, :]  # query offset within sub-block
        m = (c <= 2 * r_ + par).astype(np.float32)  # [256, 128]
        masks.append(np.ascontiguousarray(
            m.reshape(2, P, P).transpose(1, 0, 2)))

    in_maps = []
    for core in range(N_CORES):
        b, par = core // 2, core % 2
        xb = np.asarray(x[b], np.float32)
        in_maps.append({
            "xctxT": np.ascontiguousarray(xb.T),
            "xqT": np.ascontiguousarray(xb[par::2].T),
            "Wq": Wq, "Wk": Wk, "Wv": Wv, "Wp": Wp,
            "bq": bq_h, "bk": bk_h, "bp": bp_h,
            "vbias": vb_h, "maskT": masks[par],
            "onesr": np.ones((1, D), np.float32),
        })
    return in_maps


def kernel(x, mask, Wq, bq, Wk, bk, Wv, bv, Wp, bp):
    from concourse import bass_utils

    if "nc" not in _CACHE:
        _CACHE["nc"] = _build_nc()
    nc = _CACHE["nc"]

    in_maps = _prep_in_maps(x, mask, Wq, bq, Wk, bk, Wv, bv, Wp, bp)
    res = bass_utils.run_bass_kernel_spmd(
        nc, in_maps, core_ids=list(range(N_CORES)))

    out = np.empty((B, T, C), np.float32)
    for core in range(N_CORES):
        b, par = core // 2, core % 2
        out[b, par::2, :] = res.results[core]["outT"].T
    return out


# revision 24
# speedup vs baseline: 1.2061x; 1.2061x over previous
"""Causal self-attention (B=4, T=2048, C=1024, H=16) on 8 TRN2 NeuronCores.

Sharding: core = 2*b + parity. Each core handles batch b's queries at
tokens parity::2 (1024 queries). K/V are computed for the full 2048-token
context (redundantly per batch pair) so no collectives are needed, and the
even/odd interleave makes the causal block structure identical on every
core: local query sub-block m (128 queries) attends exactly key blocks
0..2m+1, with a single shared [128(key),128(query)] diagonal mask per
parity applied to the last two key blocks.

Dataflow (all transposed, zero on-chip transposes):
  xT [C, tok] --Wk--> kT [C, 2048] (f32r matmul, bf16 storage)
             --Wv--> v [tok, C(+ones col)] natural layout, bf16
  xqT --Wq--> qT [C, 1024] bf16
  S^T[keys,q] = kT_h^T @ qT_h (bf16), exp on ScalarE (no max subtraction:
  |S|/8 <~ 6), diagonal-mask mul, P bf16.
  y^T[65,q] = [v_h | 1]^T @ P: row 64 = softmax denominator for free.
  1/denom broadcast across partitions via K=1 matmul; out-proj in bf16.

Phase C runs a flat software pipeline over the 34 (query-block, head)
steps: the PE stream per step is bc(s-2), [outproj-j0 filler],
scores(s) interleaved with AV(s-1) — keeping TensorE continuously busy
(HAM stays at full clock) while ScalarE exps one head behind.
"""

import math
from contextlib import ExitStack

import numpy as np

B, T, C, H = 4, 2048, 1024, 16
D = C // H  # 64
P = 128
N_CORES = 8
NKB = T // P  # 16 key blocks of 128
TQ = T // 2  # 1024 queries per core
NQB = 2  # query blocks of 512
SCALE = 1.0 / math.sqrt(D)

_CACHE = {}


def _build_nc():
    import concourse.tile as tile
    from concourse import bacc, mybir
    from concourse.bass_interp import get_hw_module
    from concourse import hw_specs

    if not getattr(bacc, "_attn_act_tbl_patch", False):
        _orig_tables = hw_specs.get_activation_tables

        def _tables_exp_with_ln(arch):
            t = _orig_tables(arch)
            for name, fns in t.items():
                if name != "natural_log_exp_and_others":
                    fns.discard(mybir.ActivationFunctionType.Exp)
            return t

        bacc.get_activation_tables = _tables_exp_with_ln
        bacc._attn_act_tbl_patch = True

    f32 = mybir.dt.float32
    f32r = mybir.dt.float32r
    bf16 = mybir.dt.bfloat16

    nc = bacc.Bacc("TRN2", target_bir_lowering=False, debug=False,
                   num_devices=N_CORES)

    xctxT = nc.dram_tensor("xctxT", [C, T], f32r, kind="ExternalInput").ap()
    xqT = nc.dram_tensor("xqT", [C, TQ], f32r, kind="ExternalInput").ap()
    Wq = nc.dram_tensor("Wq", [C, C], f32r, kind="ExternalInput").ap()
    Wk = nc.dram_tensor("Wk", [C, C], f32r, kind="ExternalInput").ap()
    Wv = nc.dram_tensor("Wv", [C, C], f32r, kind="ExternalInput").ap()
    Wp = nc.dram_tensor("Wp", [C, C], bf16, kind="ExternalInput").ap()
    bq = nc.dram_tensor("bq", [P, C // P], f32, kind="ExternalInput").ap()
    bk = nc.dram_tensor("bk", [P, C // P], f32, kind="ExternalInput").ap()
    bp = nc.dram_tensor("bp", [P, C // P], f32, kind="ExternalInput").ap()
    vbias = nc.dram_tensor("vbias", [P, H, D], f32, kind="ExternalInput").ap()
    maskT = nc.dram_tensor("maskT", [P, 2, P], f32, kind="ExternalInput").ap()
    onesr = nc.dram_tensor("onesr", [1, D], f32r, kind="ExternalInput").ap()
    outT = nc.dram_tensor("outT", [C, TQ], f32, kind="ExternalOutput").ap()

    CB = C // P  # 8 channel blocks

    with tile.TileContext(nc) as tc, ExitStack() as top:
        persist = top.enter_context(tc.tile_pool(name="persist", bufs=1))
        small = top.enter_context(tc.tile_pool(name="small", bufs=1))

        # persistent SBUF tensors (all bf16: 81 KB/partition total)
        kT_sb = persist.tile([P, CB, T], bf16, tag="kT")
        v_sb = persist.tile([P, NKB, H, D + 1], bf16, tag="v")
        qT_sb = persist.tile([P, CB, TQ], bf16, tag="qT")

        bq_sb = small.tile([P, CB], f32, tag="bq")
        bk_sb = small.tile([P, CB], f32, tag="bk")
        bp_sb = small.tile([P, CB], f32, tag="bp")
        vb_sb = small.tile([P, H, D], f32, tag="vb")
        mask_sb = small.tile([P, 2, P], bf16, tag="mask")
        mask_f32 = small.tile([P, 2, P], f32, tag="maskf")
        ones_sb = small.tile([1, D], f32r, tag="ones")

        nc.sync.dma_start(bq_sb[:], bq[:])
        nc.sync.dma_start(bk_sb[:], bk[:])
        nc.sync.dma_start(bp_sb[:], bp[:])
        nc.sync.dma_start(vb_sb[:], vbias[:])
        nc.sync.dma_start(mask_f32[:], maskT[:])
        nc.vector.tensor_copy(mask_sb[:], mask_f32[:])
        nc.sync.dma_start(ones_sb[:], onesr[:])
        # ones column of v (AV rides the softmax denominator in row 64)
        nc.vector.memset(v_sb[:, :, :, D:D + 1], 1.0)

        def copy_bias(out, psum, bias_col):
            # PSUM -> SBUF copy + per-partition bias on the (idle) ScalarE
            nc.scalar.activation(out, psum,
                                 mybir.ActivationFunctionType.Identity,
                                 bias=bias_col)

        # ---------------- Phase A: K and V projections ----------------
        TC = 512  # token chunk
        with ExitStack() as sa:
            wpool = sa.enter_context(tc.tile_pool(name="wpool", bufs=2))
            xin = sa.enter_context(tc.tile_pool(name="xin", bufs=3))
            pmm = sa.enter_context(
                tc.tile_pool(name="pmm", bufs=2, space="PSUM"))

            wk_sb = wpool.tile([P, CB, C], f32r, tag="W")
            nc.sync.dma_start(wk_sb[:], Wk.rearrange("(o p) c -> p o c", p=P))
            wv_sb = wpool.tile([P, CB, C], f32r, tag="W")
            nc.sync.dma_start(wv_sb[:], Wv.rearrange("(o p) c -> p o c", p=P))

            for t0 in range(0, T, TC):
                x_t = xin.tile([P, CB, TC], f32r, tag="x")
                nc.sync.dma_start(
                    x_t[:],
                    xctxT[:, t0:t0 + TC].rearrange("(o p) t -> p o t", p=P))
                # K: kT rows (transposed layout)
                for rb in range(CB):
                    ps = pmm.tile([P, TC], f32, tag="mm")
                    for kc in range(CB):
                        nc.tensor.matmul(
                            ps[:], wk_sb[:, kc, rb * P:(rb + 1) * P],
                            x_t[:, kc, :], start=(kc == 0),
                            stop=(kc == CB - 1))
                    copy_bias(kT_sb[:, rb, t0:t0 + TC], ps[:],
                              bk_sb[:, rb:rb + 1])
                # V: natural layout [tok, C]
                for tb in range(TC // P):
                    kb = (t0 + tb * P) // P
                    for cb2 in range(2):
                        ps = pmm.tile([P, TC], f32, tag="mm")
                        for kc in range(CB):
                            nc.tensor.matmul(
                                ps[:], x_t[:, kc, tb * P:(tb + 1) * P],
                                wv_sb[:, kc, cb2 * 512:(cb2 + 1) * 512],
                                start=(kc == 0), stop=(kc == CB - 1))
                        h0 = cb2 * 8
                        nc.vector.tensor_tensor(
                            v_sb[:, kb, h0:h0 + 8, 0:D],
                            ps.rearrange("p (h d) -> p h d", d=D),
                            vb_sb[:, h0:h0 + 8, :], mybir.AluOpType.add)

        # ---------------- Phase B: Q projection ----------------
        with ExitStack() as sb:
            wqp = sb.enter_context(tc.tile_pool(name="wqp", bufs=3))
            xqin = sb.enter_context(tc.tile_pool(name="xqin", bufs=2))
            pmm = sb.enter_context(
                tc.tile_pool(name="pmmB", bufs=2, space="PSUM"))
            for q0 in range(0, TQ, 512):
                xq_t = xqin.tile([P, CB, 512], f32r, tag="xq")
                nc.gpsimd.dma_start(
                    xq_t[:],
                    xqT[:, q0:q0 + 512].rearrange("(o p) t -> p o t", p=P))
                for rb in range(CB):
                    wq_t = wqp.tile([P, CB, P], f32r, tag="wq")
                    nc.gpsimd.dma_start(
                        wq_t[:], Wq[:, rb * P:(rb + 1) * P].rearrange(
                            "(o p) c -> p o c", p=P))
                    ps = pmm.tile([P, 512], f32, tag="mm")
                    for kc in range(CB):
                        nc.tensor.matmul(
                            ps[:], wq_t[:, kc, :], xq_t[:, kc, :],
                            start=(kc == 0), stop=(kc == CB - 1))
                    copy_bias(qT_sb[:, rb, q0:q0 + 512], ps[:],
                              bq_sb[:, rb:rb + 1])

        # -------- Phase C: attention + output projection (flat pipeline) ----
        with ExitStack() as sc:
            ppool = sc.enter_context(tc.tile_pool(name="ppool", bufs=2))
            ypool = sc.enter_context(tc.tile_pool(name="ypool", bufs=2))
            opool = sc.enter_context(tc.tile_pool(name="opool", bufs=2))
            wpp = sc.enter_context(tc.tile_pool(name="wpp", bufs=2))
            nrm = sc.enter_context(tc.tile_pool(name="nrm", bufs=3))
            ps_s = sc.enter_context(
                tc.tile_pool(name="ps_s", bufs=2, space="PSUM"))
            ps_y = sc.enter_context(
                tc.tile_pool(name="ps_y", bufs=3, space="PSUM"))
            ps_x = sc.enter_context(
                tc.tile_pool(name="ps_x", bufs=1, space="PSUM"))

            L = [(0, h) for h in range(H)] + [(1, h) for h in range(H)]
            P_ts, py_ts, recips = {}, {}, {}
            yT_tiles = {}

            def qstart(j, kb):
                return max(0, kb // 2 - 4 * j) * P

            def emit_outproj(j, ob):
                q0 = j * 512
                yT_sb = yT_tiles[j]
                wp_t = wpp.tile([P, CB, P], bf16, tag="wp")
                nc.gpsimd.dma_start(
                    wp_t[:], Wp[:, ob * P:(ob + 1) * P].rearrange(
                        "(o p) c -> p o c", p=P))
                po = ps_x.tile([P, 512], f32, tag="bx")
                for yc in range(CB):
                    nc.tensor.matmul(po[:], wp_t[:, yc, :], yT_sb[:, yc, :],
                                     start=(yc == 0), stop=(yc == CB - 1))
                o_sb = opool.tile([P, 512], f32, tag="o_sb")
                copy_bias(o_sb[:], po[:], bp_sb[:, ob:ob + 1])
                nc.sync.dma_start(outT[ob * P:(ob + 1) * P, q0:q0 + 512],
                                  o_sb[:])

            for s in range(len(L) + 3):
                cur = L[s] if s < len(L) else None
                prv = L[s - 1] if 1 <= s <= len(L) else None
                pp2 = L[s - 3] if s >= 3 else None

                # --- PE filler: j0 output projection inside j1 steps ---
                if 19 <= s <= 26:
                    emit_outproj(0, s - 19)

                # --- scores(cur) interleaved with AV(prv) ---
                sc_pairs = []
                if cur is not None:
                    j, h = cur
                    if h == 0:
                        yT_tiles[j] = ypool.tile([P, CB, 512], bf16, tag="yT", name=f"yT{j}")
                    kmax = 8 * j + 8
                    sc_pairs = list(range(kmax // 2))
                    P_ts[cur] = ppool.tile([P, NKB, 512], bf16, tag="P", name=f"Pt{s}")
                av_kbs = []
                if prv is not None:
                    av_kbs = list(range(8 * prv[0] + 8))
                    py_ts[prv] = ps_y.tile([D + 1, 512], f32, tag="y", name=f"py{s}")

                np_, na = max(len(sc_pairs), 1), len(av_kbs)
                for i, p_ in enumerate(sc_pairs or [None]):
                    if p_ is not None:
                        j, h = cur
                        q0 = j * 512
                        hp, hb = (h % 2) * D, h // 2
                        P_t = P_ts[cur]
                        qs = qstart(j, 2 * p_)
                        ss = ps_s.tile([P, 1024], f32, tag="s")
                        for dj in range(2):
                            kb = 2 * p_ + dj
                            nc.tensor.matmul(
                                ss[:, dj * 512 + qs:(dj + 1) * 512],
                                kT_sb[hp:hp + D, hb, kb * P:(kb + 1) * P],
                                qT_sb[hp:hp + D, hb, q0 + qs:q0 + 512],
                                start=True, stop=True)
                        if qs == 0:
                            nc.scalar.activation(
                                P_t.rearrange("p a b -> p (a b)")
                                [:, 2 * p_ * 512:(2 * p_ + 2) * 512],
                                ss[:], mybir.ActivationFunctionType.Exp,
                                scale=SCALE)
                        else:
                            nc.scalar.activation(
                                P_t[:, 2 * p_:2 * p_ + 2, qs:512],
                                ss.rearrange("p (a b) -> p a b", a=2)
                                [:, :, qs:512],
                                mybir.ActivationFunctionType.Exp, scale=SCALE)
                    # AV share for this slot
                    lo = na * i // np_
                    hi = na * (i + 1) // np_
                    for kb in av_kbs[lo:hi]:
                        jj, hh = prv
                        avs = qstart(jj, kb)
                        nc.tensor.matmul(
                            py_ts[prv][:, avs:512], v_sb[:, kb, hh, :],
                            P_ts[prv][:, kb, avs:512],
                            start=(kb == 0), stop=(kb == len(av_kbs) - 1))

                # --- DVE: causal diagonal masks for cur (must precede the
                # reciprocal in the DVE FIFO so next step's AV isn't stuck
                # behind it) ---
                if cur is not None:
                    j, h = cur
                    P_t = P_ts[cur]
                    for mq in range(4):
                        kb = 2 * (4 * j + mq)
                        sl = P_t[:, kb:kb + 2, mq * P:(mq + 1) * P]
                        # on GpSimd (otherwise idle): keeps the masks off the
                        # DVE FIFO so AV(next step) isn't queued behind the
                        # reciprocal, and shortens the exp->mask->AV chain
                        nc.gpsimd.tensor_mul(sl, sl, mask_sb[:])

                if prv is not None:
                    P_ts.pop(prv)
                    recip = nrm.tile([1, 512], f32r, tag="recip")
                    recips[prv] = recip
                    if prv[0] == 1:
                        with nc.allow_low_precision(
                                reason="f32r (12-bit) softmax denominators"):
                            nc.vector.reciprocal(recip[:],
                                                 py_ts[prv][D:D + 1, :])
                    else:
                        lnd = nrm.tile([1, 512], f32, tag="lnd")
                        nc.scalar.activation(
                            lnd[:], py_ts[prv][D:D + 1, :],
                            mybir.ActivationFunctionType.Ln)
                        nc.scalar.activation(
                            recip[:], lnd[:],
                            mybir.ActivationFunctionType.Exp, scale=-1.0)

                # --- step tail: normalize head s-2 (reciprocal had a full
                # step of slack; bc sits at the end of the PE stream) ---
                if pp2 is not None:
                    jj, hh = pp2
                    bc = ps_x.tile([P, 512], f32, tag="bx",
                                   name=f"bc{s}")[0:D, :]
                    nc.tensor.matmul(bc[:], ones_sb[:], recips.pop(pp2)[:],
                                     start=True, stop=True)
                    bc_sb = nrm.tile([D, 512], f32, tag="bc_sb")
                    nc.vector.tensor_copy(bc_sb[:], bc[:])
                    py = py_ts.pop(pp2)
                    hp, hb = (hh % 2) * D, hh // 2
                    nc.vector.tensor_mul(yT_tiles[jj][hp:hp + D, hb, :],
                                         py[0:D, :], bc_sb[:])

            for ob in range(CB):
                emit_outproj(1, ob)

    nc.compile()
    nc.m = get_hw_module(nc.m)
    return nc


def _prep_in_maps(x, mask, Wq, bq, Wk, bk, Wv, bv, Wp, bp):
    import ml_dtypes

    del mask  # causal structure is hardcoded (tril), verified upstream
    CB = C // P
    Wq, Wk, Wv = (np.ascontiguousarray(w, np.float32) for w in (Wq, Wk, Wv))
    Wp = np.ascontiguousarray(np.asarray(Wp, np.float32).astype(
        ml_dtypes.bfloat16))
    b_col = lambda b: np.ascontiguousarray(
        np.asarray(b, np.float32).reshape(CB, P).T)
    bq_h, bk_h, bp_h = b_col(bq), b_col(bk), b_col(bp)
    vb_h = np.ascontiguousarray(np.broadcast_to(
        np.asarray(bv, np.float32).reshape(1, H, D), (P, H, D)))

    masks = []
    for par in range(2):
        c = np.arange(2 * P)[:, None]  # key offset within diagonal pair
        r_ = np.arange(P)[None, :]  # query offset within sub-block
        m = (c <= 2 * r_ + par).astype(np.float32)  # [256, 128]
        masks.append(np.ascontiguousarray(
            m.reshape(2, P, P).transpose(1, 0, 2)))

    in_maps = []
    for core in range(N_CORES):
        b, par = core // 2, core % 2
        xb = np.asarray(x[b], np.float32)
        in_maps.append({
            "xctxT": np.ascontiguousarray(xb.T),
            "xqT": np.ascontiguousarray(xb[par::2].T),
            "Wq": Wq, "Wk": Wk, "Wv": Wv, "Wp": Wp,
            "bq": bq_h, "bk": bk_h, "bp": bp_h,
            "vbias": vb_h, "maskT": masks[par],
            "onesr": np.ones((1, D), np.float32),
        })
    return in_maps


def kernel(x, mask, Wq, bq, Wk, bk, Wv, bv, Wp, bp):
    from concourse import bass_utils

    if "nc" not in _CACHE:
        _CACHE["nc"] = _build_nc()
    nc = _CACHE["nc"]

    in_maps = _prep_in_maps(x, mask, Wq, bq, Wk, bk, Wv, bv, Wp, bp)
    res = bass_utils.run_bass_kernel_spmd(
        nc, in_maps, core_ids=list(range(N_CORES)))

    out = np.empty((B, T, C), np.float32)
    for core in range(N_CORES):
        b, par = core // 2, core % 2
        out[b, par::2, :] = res.results[core]["outT"].T
    return out


# revision 25
# speedup vs baseline: 1.2816x; 1.0626x over previous
"""Causal self-attention (B=4, T=2048, C=1024, H=16) on 8 TRN2 NeuronCores.

Sharding: core = 2*b + parity. Each core handles batch b's queries at
tokens parity::2 (1024 queries). K/V are computed for the full 2048-token
context (redundantly per batch pair) so no collectives are needed, and the
even/odd interleave makes the causal block structure identical on every
core: local query sub-block m (128 queries) attends exactly key blocks
0..2m+1, with a single shared [128(key),128(query)] diagonal mask per
parity applied to the last two key blocks.

Dataflow (all transposed, zero on-chip transposes):
  xT [C, tok] --Wk--> kT [C, 2048] (f32r matmul, bf16 storage)
             --Wv--> v [tok, C(+ones col)] natural layout, bf16
  xqT --Wq--> qT [C, 1024] bf16
  S^T[keys,q] = kT_h^T @ qT_h (bf16), exp on ScalarE (no max subtraction:
  |S|/8 <~ 6), diagonal-mask mul, P bf16.
  y^T[65,q] = [v_h | 1]^T @ P: row 64 = softmax denominator for free.
  1/denom broadcast across partitions via K=1 matmul; out-proj in bf16.

Phase C runs a flat software pipeline over the 34 (query-block, head)
steps: the PE stream per step is bc(s-2), [outproj-j0 filler],
scores(s) interleaved with AV(s-1) — keeping TensorE continuously busy
(HAM stays at full clock) while ScalarE exps one head behind.
"""

import math
from contextlib import ExitStack

import numpy as np

B, T, C, H = 4, 2048, 1024, 16
D = C // H  # 64
P = 128
N_CORES = 8
NKB = T // P  # 16 key blocks of 128
TQ = T // 2  # 1024 queries per core
NQB = 2  # query blocks of 512
SCALE = 1.0 / math.sqrt(D)

_CACHE = {}


def _build_nc():
    import concourse.tile as tile
    from concourse import bacc, mybir
    from concourse.bass_interp import get_hw_module
    from concourse import hw_specs

    if not getattr(bacc, "_attn_act_tbl_patch", False):
        _orig_tables = hw_specs.get_activation_tables

        def _tables_exp_with_ln(arch):
            t = _orig_tables(arch)
            for name, fns in t.items():
                if name != "natural_log_exp_and_others":
                    fns.discard(mybir.ActivationFunctionType.Exp)
            return t

        bacc.get_activation_tables = _tables_exp_with_ln
        bacc._attn_act_tbl_patch = True

    f32 = mybir.dt.float32
    f32r = mybir.dt.float32r
    bf16 = mybir.dt.bfloat16

    nc = bacc.Bacc("TRN2", target_bir_lowering=False, debug=False,
                   num_devices=N_CORES)

    xctxT = nc.dram_tensor("xctxT", [C, T], f32r, kind="ExternalInput").ap()
    xqT = nc.dram_tensor("xqT", [C, TQ], f32r, kind="ExternalInput").ap()
    Wq = nc.dram_tensor("Wq", [C, C], f32r, kind="ExternalInput").ap()
    Wk = nc.dram_tensor("Wk", [C, C], f32r, kind="ExternalInput").ap()
    Wv = nc.dram_tensor("Wv", [C, C], f32r, kind="ExternalInput").ap()
    Wp = nc.dram_tensor("Wp", [C, C], bf16, kind="ExternalInput").ap()
    bq = nc.dram_tensor("bq", [P, C // P], f32, kind="ExternalInput").ap()
    bk = nc.dram_tensor("bk", [P, C // P], f32, kind="ExternalInput").ap()
    bp = nc.dram_tensor("bp", [P, C // P], f32, kind="ExternalInput").ap()
    vbias = nc.dram_tensor("vbias", [P, H, D], f32, kind="ExternalInput").ap()
    maskT = nc.dram_tensor("maskT", [P, 2, P], f32, kind="ExternalInput").ap()
    onesr = nc.dram_tensor("onesr", [1, D], f32r, kind="ExternalInput").ap()
    outT = nc.dram_tensor("outT", [C, TQ], f32, kind="ExternalOutput").ap()

    CB = C // P  # 8 channel blocks

    with tile.TileContext(nc) as tc, ExitStack() as top:
        persist = top.enter_context(tc.tile_pool(name="persist", bufs=1))
        small = top.enter_context(tc.tile_pool(name="small", bufs=1))

        # persistent SBUF tensors (all bf16: 81 KB/partition total)
        kT_sb = persist.tile([P, CB, T], bf16, tag="kT")
        v_sb = persist.tile([P, NKB, H, D + 1], bf16, tag="v")
        qT_sb = persist.tile([P, CB, TQ], bf16, tag="qT")

        bq_sb = small.tile([P, CB], f32, tag="bq")
        bk_sb = small.tile([P, CB], f32, tag="bk")
        bp_sb = small.tile([P, CB], f32, tag="bp")
        vb_sb = small.tile([P, H, D], f32, tag="vb")
        mask_sb = small.tile([P, 2, P], bf16, tag="mask")
        mask_f32 = small.tile([P, 2, P], f32, tag="maskf")
        ones_sb = small.tile([1, D], f32r, tag="ones")

        nc.sync.dma_start(bq_sb[:], bq[:])
        nc.sync.dma_start(bk_sb[:], bk[:])
        nc.sync.dma_start(bp_sb[:], bp[:])
        nc.sync.dma_start(vb_sb[:], vbias[:])
        nc.sync.dma_start(mask_f32[:], maskT[:])
        nc.vector.tensor_copy(mask_sb[:], mask_f32[:])
        nc.sync.dma_start(ones_sb[:], onesr[:])
        # ones column of v (AV rides the softmax denominator in row 64)
        nc.vector.memset(v_sb[:, :, :, D:D + 1], 1.0)

        def copy_bias(out, psum, bias_col):
            # PSUM -> SBUF copy + per-partition bias on the (idle) ScalarE
            nc.scalar.activation(out, psum,
                                 mybir.ActivationFunctionType.Identity,
                                 bias=bias_col)

        # ---------------- Phase A: K and V projections ----------------
        TC = 512  # token chunk
        with ExitStack() as sa:
            wpool = sa.enter_context(tc.tile_pool(name="wpool", bufs=2))
            xin = sa.enter_context(tc.tile_pool(name="xin", bufs=3))
            pmm = sa.enter_context(
                tc.tile_pool(name="pmm", bufs=2, space="PSUM"))

            wk_sb = wpool.tile([P, CB, C], f32r, tag="W")
            nc.sync.dma_start(wk_sb[:], Wk.rearrange("(o p) c -> p o c", p=P))
            wv_sb = wpool.tile([P, CB, C], f32r, tag="W")
            nc.sync.dma_start(wv_sb[:], Wv.rearrange("(o p) c -> p o c", p=P))

            for t0 in range(0, T, TC):
                x_t = xin.tile([P, CB, TC], f32r, tag="x")
                nc.sync.dma_start(
                    x_t[:],
                    xctxT[:, t0:t0 + TC].rearrange("(o p) t -> p o t", p=P))
                # K: kT rows (transposed layout)
                for rb in range(CB):
                    ps = pmm.tile([P, TC], f32, tag="mm")
                    for kc in range(CB):
                        nc.tensor.matmul(
                            ps[:], wk_sb[:, kc, rb * P:(rb + 1) * P],
                            x_t[:, kc, :], start=(kc == 0),
                            stop=(kc == CB - 1))
                    copy_bias(kT_sb[:, rb, t0:t0 + TC], ps[:],
                              bk_sb[:, rb:rb + 1])
                # V: natural layout [tok, C]
                for tb in range(TC // P):
                    kb = (t0 + tb * P) // P
                    for cb2 in range(2):
                        ps = pmm.tile([P, TC], f32, tag="mm")
                        for kc in range(CB):
                            nc.tensor.matmul(
                                ps[:], x_t[:, kc, tb * P:(tb + 1) * P],
                                wv_sb[:, kc, cb2 * 512:(cb2 + 1) * 512],
                                start=(kc == 0), stop=(kc == CB - 1))
                        h0 = cb2 * 8
                        nc.vector.tensor_tensor(
                            v_sb[:, kb, h0:h0 + 8, 0:D],
                            ps.rearrange("p (h d) -> p h d", d=D),
                            vb_sb[:, h0:h0 + 8, :], mybir.AluOpType.add)

        # ---------------- Phase B: Q projection ----------------
        with ExitStack() as sb:
            wqp = sb.enter_context(tc.tile_pool(name="wqp", bufs=3))
            xqin = sb.enter_context(tc.tile_pool(name="xqin", bufs=2))
            pmm = sb.enter_context(
                tc.tile_pool(name="pmmB", bufs=2, space="PSUM"))
            for q0 in range(0, TQ, 512):
                xq_t = xqin.tile([P, CB, 512], f32r, tag="xq")
                nc.gpsimd.dma_start(
                    xq_t[:],
                    xqT[:, q0:q0 + 512].rearrange("(o p) t -> p o t", p=P))
                for rb in range(CB):
                    wq_t = wqp.tile([P, CB, P], f32r, tag="wq")
                    nc.gpsimd.dma_start(
                        wq_t[:], Wq[:, rb * P:(rb + 1) * P].rearrange(
                            "(o p) c -> p o c", p=P))
                    ps = pmm.tile([P, 512], f32, tag="mm")
                    for kc in range(CB):
                        nc.tensor.matmul(
                            ps[:], wq_t[:, kc, :], xq_t[:, kc, :],
                            start=(kc == 0), stop=(kc == CB - 1))
                    copy_bias(qT_sb[:, rb, q0:q0 + 512], ps[:],
                              bq_sb[:, rb:rb + 1])

        # -------- Phase C: attention + output projection (flat pipeline) ----
        with ExitStack() as sc:
            ppool = sc.enter_context(tc.tile_pool(name="ppool", bufs=2))
            ypool = sc.enter_context(tc.tile_pool(name="ypool", bufs=2))
            opool = sc.enter_context(tc.tile_pool(name="opool", bufs=2))
            wpp = sc.enter_context(tc.tile_pool(name="wpp", bufs=2))
            nrm = sc.enter_context(tc.tile_pool(name="nrm", bufs=3))
            ps_s = sc.enter_context(
                tc.tile_pool(name="ps_s", bufs=2, space="PSUM"))
            ps_y = sc.enter_context(
                tc.tile_pool(name="ps_y", bufs=3, space="PSUM"))
            ps_x = sc.enter_context(
                tc.tile_pool(name="ps_x", bufs=1, space="PSUM"))

            L = [(0, h) for h in range(H)] + [(1, h) for h in range(H)]
            P_ts, py_ts, recips = {}, {}, {}
            yT_tiles = {}

            def qstart(j, kb):
                return max(0, kb // 2 - 4 * j) * P

            def emit_outproj(j, ob, half=None):
                q0 = j * 512
                NQO = 512 if half is None else 256
                if half:
                    q0 += 256
                yT_sb = yT_tiles[j]
                wp_t = wpp.tile([P, CB, P], bf16, tag="wp")
                nc.gpsimd.dma_start(
                    wp_t[:], Wp[:, ob * P:(ob + 1) * P].rearrange(
                        "(o p) c -> p o c", p=P))
                po = ps_x.tile([P, 512], f32, tag="bx")
                for yc in range(CB):
                    nc.tensor.matmul(po[:, :NQO], wp_t[:, yc, :],
                                     yT_sb[:, yc, q0 - j * 512:
                                           q0 - j * 512 + NQO],
                                     start=(yc == 0), stop=(yc == CB - 1))
                o_sb = opool.tile([P, 512], f32, tag="o_sb")
                copy_bias(o_sb[:, :NQO], po[:, :NQO], bp_sb[:, ob:ob + 1])
                nc.sync.dma_start(outT[ob * P:(ob + 1) * P, q0:q0 + NQO],
                                  o_sb[:, :NQO])

            for s in range(len(L) + 3):
                cur = L[s] if s < len(L) else None
                prv = L[s - 1] if 1 <= s <= len(L) else None
                pp2 = L[s - 3] if s >= 3 else None

                # --- PE filler: j0 output projection inside j1 steps ---
                # j0 outproj split into 16 half-width groups spread
                # over every j1 step (PE filler + HAM continuity)
                if 19 <= s <= 31:
                    hg = s - 19
                    emit_outproj(0, hg % CB, hg // CB)
                    if s >= 29:
                        hg = 13 + (s - 29)
                        emit_outproj(0, hg % CB, hg // CB)

                # --- scores(cur) interleaved with AV(prv) ---
                sc_pairs = []
                if cur is not None:
                    j, h = cur
                    if h == 0:
                        yT_tiles[j] = ypool.tile([P, CB, 512], bf16, tag="yT", name=f"yT{j}")
                    kmax = 8 * j + 8
                    sc_pairs = list(range(kmax // 2))
                    P_ts[cur] = ppool.tile([P, NKB, 512], bf16, tag="P", name=f"Pt{s}")
                av_kbs = []
                if prv is not None:
                    av_kbs = list(range(8 * prv[0] + 8))
                    py_ts[prv] = ps_y.tile([D + 1, 512], f32, tag="y", name=f"py{s}")

                np_, na = max(len(sc_pairs), 1), len(av_kbs)
                for i, p_ in enumerate(sc_pairs or [None]):
                    if p_ is not None:
                        j, h = cur
                        q0 = j * 512
                        hp, hb = (h % 2) * D, h // 2
                        P_t = P_ts[cur]
                        qs = qstart(j, 2 * p_)
                        ss = ps_s.tile([P, 1024], f32, tag="s")
                        for dj in range(2):
                            kb = 2 * p_ + dj
                            nc.tensor.matmul(
                                ss[:, dj * 512 + qs:(dj + 1) * 512],
                                kT_sb[hp:hp + D, hb, kb * P:(kb + 1) * P],
                                qT_sb[hp:hp + D, hb, q0 + qs:q0 + 512],
                                start=True, stop=True)
                        if qs == 0:
                            nc.scalar.activation(
                                P_t.rearrange("p a b -> p (a b)")
                                [:, 2 * p_ * 512:(2 * p_ + 2) * 512],
                                ss[:], mybir.ActivationFunctionType.Exp,
                                scale=SCALE)
                        else:
                            nc.scalar.activation(
                                P_t[:, 2 * p_:2 * p_ + 2, qs:512],
                                ss.rearrange("p (a b) -> p a b", a=2)
                                [:, :, qs:512],
                                mybir.ActivationFunctionType.Exp, scale=SCALE)
                    # AV share for this slot
                    lo = na * i // np_
                    hi = na * (i + 1) // np_
                    for kb in av_kbs[lo:hi]:
                        jj, hh = prv
                        avs = qstart(jj, kb)
                        nc.tensor.matmul(
                            py_ts[prv][:, avs:512], v_sb[:, kb, hh, :],
                            P_ts[prv][:, kb, avs:512],
                            start=(kb == 0), stop=(kb == len(av_kbs) - 1))

                # --- DVE: causal diagonal masks for cur (must precede the
                # reciprocal in the DVE FIFO so next step's AV isn't stuck
                # behind it) ---
                if cur is not None:
                    j, h = cur
                    P_t = P_ts[cur]
                    for mq in range(4):
                        kb = 2 * (4 * j + mq)
                        sl = P_t[:, kb:kb + 2, mq * P:(mq + 1) * P]
                        # on GpSimd (otherwise idle): keeps the masks off the
                        # DVE FIFO so AV(next step) isn't queued behind the
                        # reciprocal, and shortens the exp->mask->AV chain
                        nc.gpsimd.tensor_mul(sl, sl, mask_sb[:])

                if prv is not None:
                    P_ts.pop(prv)
                    recip = nrm.tile([1, 512], f32r, tag="recip")
                    recips[prv] = recip
                    if prv[0] == 1:
                        with nc.allow_low_precision(
                                reason="f32r (12-bit) softmax denominators"):
                            nc.vector.reciprocal(recip[:],
                                                 py_ts[prv][D:D + 1, :])
                    else:
                        lnd = nrm.tile([1, 512], f32, tag="lnd")
                        nc.scalar.activation(
                            lnd[:], py_ts[prv][D:D + 1, :],
                            mybir.ActivationFunctionType.Ln)
                        nc.scalar.activation(
                            recip[:], lnd[:],
                            mybir.ActivationFunctionType.Exp, scale=-1.0)

                # --- step tail: normalize head s-2 (reciprocal had a full
                # step of slack; bc sits at the end of the PE stream) ---
                if pp2 is not None:
                    jj, hh = pp2
                    bc = ps_x.tile([P, 512], f32, tag="bx",
                                   name=f"bc{s}")[0:D, :]
                    nc.tensor.matmul(bc[:], ones_sb[:], recips.pop(pp2)[:],
                                     start=True, stop=True)
                    bc_sb = nrm.tile([D, 512], f32, tag="bc_sb")
                    nc.vector.tensor_copy(bc_sb[:], bc[:])
                    py = py_ts.pop(pp2)
                    hp, hb = (hh % 2) * D, hh // 2
                    nc.vector.tensor_mul(yT_tiles[jj][hp:hp + D, hb, :],
                                         py[0:D, :], bc_sb[:])

            for ob in range(CB):
                emit_outproj(1, ob)

    nc.compile()
    nc.m = get_hw_module(nc.m)
    return nc


def _prep_in_maps(x, mask, Wq, bq, Wk, bk, Wv, bv, Wp, bp):
    import ml_dtypes

    del mask  # causal structure is hardcoded (tril), verified upstream
    CB = C // P
    Wq, Wk, Wv = (np.ascontiguousarray(w, np.float32) for w in (Wq, Wk, Wv))
    Wp = np.ascontiguousarray(np.asarray(Wp, np.float32).astype(
        ml_dtypes.bfloat16))
    b_col = lambda b: np.ascontiguousarray(
        np.asarray(b, np.float32).reshape(CB, P).T)
    bq_h, bk_h, bp_h = b_col(bq), b_col(bk), b_col(bp)
    vb_h = np.ascontiguousarray(np.broadcast_to(
        np.asarray(bv, np.float32).reshape(1, H, D), (P, H, D)))

    masks = []
    for par in range(2):
        c = np.arange(2 * P)[:, None]  # key offset within diagonal pair
        r_ = np.arange(P)[None, :]  # query offset within sub-block
        m = (c <= 2 * r_ + par).astype(np.float32)  # [256, 128]
        masks.append(np.ascontiguousarray(
            m.reshape(2, P, P).transpose(1, 0, 2)))

    in_maps = []
    for core in range(N_CORES):
        b, par = core // 2, core % 2
        xb = np.asarray(x[b], np.float32)
        in_maps.append({
            "xctxT": np.ascontiguousarray(xb.T),
            "xqT": np.ascontiguousarray(xb[par::2].T),
            "Wq": Wq, "Wk": Wk, "Wv": Wv, "Wp": Wp,
            "bq": bq_h, "bk": bk_h, "bp": bp_h,
            "vbias": vb_h, "maskT": masks[par],
            "onesr": np.ones((1, D), np.float32),
        })
    return in_maps


def kernel(x, mask, Wq, bq, Wk, bk, Wv, bv, Wp, bp):
    from concourse import bass_utils

    if "nc" not in _CACHE:
        _CACHE["nc"] = _build_nc()
    nc = _CACHE["nc"]

    in_maps = _prep_in_maps(x, mask, Wq, bq, Wk, bk, Wv, bv, Wp, bp)
    res = bass_utils.run_bass_kernel_spmd(
        nc, in_maps, core_ids=list(range(N_CORES)))

    out = np.empty((B, T, C), np.float32)
    for core in range(N_CORES):
        b, par = core // 2, core % 2
        out[b, par::2, :] = res.results[core]["outT"].T
    return out
